# revision 37
# baseline (speedup 1.0000x reference)
"""CAMMambaBlock Trainium2 kernel, v2 (state-interleaved scan layout).

Data-parallel over batch: 8 batch elements -> 8 NeuronCores. Each core runs
the full block (LayerNorm -> in_proj -> causal depthwise conv -> SiLU ->
x_proj -> dt softplus -> selective scan -> gating -> out_proj -> residual)
on its own (c=128, L=9216) slice, streaming over L in chunks of 1536.

Key layout: the selective scan runs in a state-interleaved layout.  For each
of 16 channel-groups g (8 channels each), a [128, Tc] tile holds all 16 SSM
states: partition p = n*8 + c8 carries the recurrence for (state n, channel
8g+c8).  This makes the per-state B/C coefficient broadcast a single shared
replicated tile per chunk (instead of 16 per-state broadcasts), and turns
the sum over states into PE selector-matmuls that accumulate in PSUM
(instead of a DVE/GpSimd reduction tree).

GpSimd is parked (it shares the DVE's second SBUF port; the v1 kernel's
heavy GpSimd elementwise load degraded DVE scans/TTs by 1.2-2x).  All
elementwise work is DVE in bf16 (2x mode); transcendentals on Scalar;
reductions and projections on the PE.  LayerNorm's ln_w/ln_b and the conv
bias are folded into the in_proj weights host-side.
"""
import types
import numpy as np
import ml_dtypes
from contextlib import ExitStack

import bass_rust

import concourse.bass as bass
import concourse.bacc as bacc
import concourse.tile as tile
from concourse import mybir
from concourse.bass_utils import run_bass_kernel_spmd
from concourse.hw_specs import get_activation_tables


def _single_act_table(self):
    """Force every activation onto natural_log_exp_and_others so the
    table-load pass hoists to one load."""
    if not any(i.opcode == "Activation" for i in self.all_instructions()):
        return
    keep = "natural_log_exp_and_others"
    tables = [(n, (f if n == keep else set()))
              for n, f in get_activation_tables(self.m.arch).items()]
    bass_rust.insert_act_table_loads(self, tables)


F32 = mybir.dt.float32
BF16 = mybir.dt.bfloat16
AF = mybir.ActivationFunctionType
OP = mybir.AluOpType

C = 128           # channels == d_inner == partitions
NST = 16          # SSM state dim
NG = 16           # channel groups of 8
RANK = 8          # dt rank
LN_EPS = 1e-5
DCONV = 4
HALO = 4          # halo columns at the left of `un` (col 0 unused, 1..3 conv)

L_FULL = 96 * 96  # 9216

# cols layout: [0]=dt_b [1]=D [2]=cbx [3]=-cbx [4]=zb [5]=-zb [6]=eps
I_DTB, I_D, I_CBX, I_MCBX, I_ZB, I_MZB, I_EPS = range(7)


def build_nc(L, Tc, sub=512):
    assert L % Tc == 0 and Tc % sub == 0
    nchunk = L // Tc
    nsub = Tc // sub

    nc = bacc.Bacc()
    x_in = nc.declare_dram_parameter("x", [C, L], F32, isOutput=False)
    w_inT = nc.declare_dram_parameter("w_inT", [C, 5 * C], BF16, isOutput=False)
    w_xpT = nc.declare_dram_parameter("w_xpT", [C, RANK + 2 * NST], BF16,
                                      isOutput=False)
    w_dtT = nc.declare_dram_parameter("w_dtT", [RANK, C], BF16, isOutput=False)
    w_outT = nc.declare_dram_parameter("w_outT", [C, C], BF16, isOutput=False)
    w_sel = nc.declare_dram_parameter("w_sel", [C, NG * C], BF16,
                                      isOutput=False)
    w_diag = nc.declare_dram_parameter("w_diag", [C, C], BF16, isOutput=False)
    cols = nc.declare_dram_parameter("cols", [C, 7], F32, isOutput=False)
    a_icols = nc.declare_dram_parameter("a_icols", [C, NG], F32,
                                        isOutput=False)
    y_out = nc.declare_dram_parameter("y", [C, L], F32, isOutput=True)

    with tile.TileContext(nc) as tc, ExitStack() as ctx:
        wpool = ctx.enter_context(tc.tile_pool(name="weights", bufs=1))
        state = ctx.enter_context(tc.tile_pool(name="state", bufs=1))
        io = ctx.enter_context(tc.tile_pool(name="io", bufs=2))
        work = ctx.enter_context(tc.tile_pool(name="work", bufs=2))
        scr = ctx.enter_context(tc.tile_pool(name="scratch", bufs=2))
        reps = ctx.enter_context(tc.tile_pool(name="reps", bufs=6))
        scanp = ctx.enter_context(tc.tile_pool(name="scan", bufs=3))
        hp = ctx.enter_context(tc.tile_pool(name="hp", bufs=5))
        bbp = ctx.enter_context(tc.tile_pool(name="bb", bufs=2))
        dram = ctx.enter_context(tc.tile_pool(name="dram", bufs=2,
                                              space="DRAM"))
        ps_st = ctx.enter_context(tc.tile_pool(name="ps_st", bufs=1,
                                               space="PSUM"))
        ps_mm = ctx.enter_context(tc.tile_pool(name="ps_mm", bufs=1,
                                               space="PSUM"))
        ps_y = ctx.enter_context(tc.tile_pool(name="ps_y", bufs=1,
                                              space="PSUM"))

        # ---- weights to SBUF (once) ----
        winT = wpool.tile([C, 5 * C], BF16, tag="winT")
        nc.sync.dma_start(winT[:], w_inT[:])
        wxpT = wpool.tile([C, RANK + 2 * NST], BF16, tag="wxpT")
        nc.sync.dma_start(wxpT[:], w_xpT[:])
        wdtT = wpool.tile([RANK, C], BF16, tag="wdtT")
        nc.sync.dma_start(wdtT[:], w_dtT[:])
        woutT = wpool.tile([C, C], BF16, tag="woutT")
        nc.sync.dma_start(woutT[:], w_outT[:])
        wsel = wpool.tile([C, NG * C], BF16, tag="wsel")
        nc.sync.dma_start(wsel[:], w_sel[:])
        wdiag = wpool.tile([C, C], BF16, tag="wdiag")
        nc.sync.dma_start(wdiag[:], w_diag[:])
        colsb = wpool.tile([C, 7], F32, tag="cols")
        nc.sync.dma_start(colsb[:], cols[:])
        aicol = wpool.tile([C, NG], F32, tag="aicol")
        nc.sync.dma_start(aicol[:], a_icols[:])
        ones_c = wpool.tile([C, C], BF16, tag="ones")
        nc.gpsimd.memset(ones_c[:], 1.0 / C)

        dt_b = colsb[:, I_DTB:I_DTB + 1]
        d_col = colsb[:, I_D:I_D + 1]
        cbx = colsb[:, I_CBX:I_CBX + 1]
        mcbx = colsb[:, I_MCBX:I_MCBX + 1]
        zb = colsb[:, I_ZB:I_ZB + 1]
        mzb = colsb[:, I_MZB:I_MZB + 1]
        eps = colsb[:, I_EPS:I_EPS + 1]

        # ---- persistent scan carries, one per channel-group ----
        carries = []
        for g in range(NG):
            cr = state.tile([C, 1], BF16, tag=f"carry{g}")
            carries.append(cr)

        ck = {}        # per-chunk live tiles, keyed by chunk index
        prev_un = [None]

        def pre(k):
            """LN -> in_proj+conv -> silu -> x_proj -> dt/v -> DRAM bounce."""
            t0 = k * Tc
            xinf = io.tile([C, Tc], F32, tag="xinf")
            nc.sync.dma_start(xinf[:], x_in[:, t0:t0 + Tc])
            xinbf = io.tile([C, Tc], BF16, tag="xinbf")
            nc.scalar.copy(xinbf[:], xinf[:])

            sq = scr.tile([C, Tc], BF16, tag="sq")
            nc.scalar.activation(sq[:], xinbf[:], AF.Square)
            un = work.tile([C, Tc + HALO], BF16, tag="un")
            if k == 0:
                nc.vector.memset(un[:, 1:HALO], 0.0)
            else:
                nc.vector.tensor_copy(un[:, 1:HALO],
                                      prev_un[0][:, Tc + 1:Tc + HALO])
            prev_un[0] = un
            for j in range(nsub):
                sl = slice(j * sub, (j + 1) * sub)
                mu = ps_st.tile([C, sub], F32, tag="mu")
                nc.tensor.matmul(mu[:], ones_c[:], xinbf[:, sl],
                                 start=True, stop=True)
                m2 = ps_st.tile([C, sub], F32, tag="m2")
                nc.tensor.matmul(m2[:], ones_c[:], sq[:, sl],
                                 start=True, stop=True)
                mubf = scr.tile([C, sub], BF16, tag="mubf")
                nc.scalar.copy(mubf[:], mu[:])
                musq = scr.tile([C, sub], F32, tag="musq")
                nc.scalar.activation(musq[:], mu[:], AF.Square)
                var = scr.tile([C, sub], F32, tag="var")
                nc.vector.tensor_tensor(var[:], m2[:], musq[:], OP.subtract)
                lnv = scr.tile([C, sub], F32, tag="lnv")
                nc.scalar.activation(lnv[:], var[:], AF.Ln, bias=eps)
                rstd = scr.tile([C, sub], BF16, tag="rstd")
                nc.scalar.activation(rstd[:], lnv[:], AF.Exp, scale=-0.5)
                dmu = scr.tile([C, sub], BF16, tag="dmu")
                nc.vector.tensor_tensor(dmu[:], xinbf[:, sl], mubf[:],
                                        OP.subtract)
                nc.vector.tensor_tensor(
                    un[:, HALO + j * sub:HALO + (j + 1) * sub],
                    dmu[:], rstd[:], OP.mult)

            zs = work.tile([C, Tc], BF16, tag="zs")
            xs = work.tile([C, Tc], BF16, tag="xs")
            for j in range(nsub):
                sl = slice(j * sub, (j + 1) * sub)
                xm_ps = ps_mm.tile([C, sub], F32, tag="mmA")
                base = HALO - (DCONV - 1) + j * sub
                for kk in range(DCONV):
                    nc.tensor.matmul(
                        xm_ps[:], winT[:, kk * C:(kk + 1) * C],
                        un[:, base + kk:base + kk + sub],
                        start=(kk == 0), stop=(kk == DCONV - 1))
                z_ps = ps_mm.tile([C, sub], F32, tag="mmB")
                nc.tensor.matmul(z_ps[:], winT[:, 4 * C:5 * C],
                                 un[:, HALO + j * sub:HALO + j * sub + sub],
                                 start=True, stop=True)
                # silu(z+zb): sigmoid via exp/ln1p/exp chain, then mult
                es1 = scr.tile([C, sub], F32, tag="es1")
                nc.scalar.activation(es1[:], z_ps[:], AF.Exp, scale=-1.0,
                                     bias=mzb)
                es2 = scr.tile([C, sub], F32, tag="es2")
                nc.scalar.activation(es2[:], es1[:], AF.Ln, bias=1.0)
                sgz = scr.tile([C, sub], BF16, tag="sgz")
                nc.scalar.activation(sgz[:], es2[:], AF.Exp, scale=-1.0)
                sz = scr.tile([C, sub], BF16, tag="sz")
                nc.scalar.activation(sz[:], z_ps[:], AF.Identity, bias=zb)
                nc.vector.tensor_tensor(zs[:, sl], sz[:], sgz[:], OP.mult)
                # silu(conv + cbx)
                ec1 = scr.tile([C, sub], F32, tag="ec1")
                nc.scalar.activation(ec1[:], xm_ps[:], AF.Exp, scale=-1.0,
                                     bias=mcbx)
                ec2 = scr.tile([C, sub], F32, tag="ec2")
                nc.scalar.activation(ec2[:], ec1[:], AF.Ln, bias=1.0)
                sgc = scr.tile([C, sub], BF16, tag="sgc")
                nc.scalar.activation(sgc[:], ec2[:], AF.Exp, scale=-1.0)
                sx = scr.tile([C, sub], BF16, tag="sx")
                nc.scalar.activation(sx[:], xm_ps[:], AF.Identity, bias=cbx)
                nc.vector.tensor_tensor(xs[:, sl], sx[:], sgc[:], OP.mult)

            bc = work.tile([2 * NST, Tc], BF16, tag="bc")
            dtr = work.tile([RANK, Tc], BF16, tag="dtr")
            for j in range(nsub):
                sl = slice(j * sub, (j + 1) * sub)
                dblf = ps_mm.tile([C, sub], F32, tag="mmB")
                dbl = dblf[0:RANK + 2 * NST, :]
                nc.tensor.matmul(dbl, wxpT[:], xs[:, sl],
                                 start=True, stop=True)
                nc.scalar.copy(bc[:, sl], dblf[0:2 * NST, :])
                nc.scalar.copy(dtr[:, sl], dblf[2 * NST:2 * NST + RANK, :])
            bc_d = dram.tile([2 * NST, Tc], BF16, tag="bc_d")
            nc.scalar.dma_start(bc_d[:], bc[:])
            bB = bbp.tile([C, Tc], BF16, tag="bB")
            nc.scalar.dma_start(
                bB[:], bc_d[0:NST, :].unsqueeze(1).broadcast_to([NST, 8, Tc]))
            bC = bbp.tile([C, Tc], BF16, tag="bC")
            nc.scalar.dma_start(
                bC[:],
                bc_d[NST:2 * NST, :].unsqueeze(1).broadcast_to([NST, 8, Tc]))

            # dt = softplus(dt_proj @ dtr + dt_b); v = dt * xs; both into one
            # DRAM scratch [C, 2Tc] so each group needs a single replica DMA
            dt_bf = work.tile([C, Tc], BF16, tag="dt")
            v_bf = work.tile([C, Tc], BF16, tag="v")
            dtv_d = dram.tile([C, 2 * Tc], BF16, tag="dtv_d")
            for j in range(nsub):
                sl = slice(j * sub, (j + 1) * sub)
                dt_ps = ps_mm.tile([C, sub], F32, tag="mmA")
                nc.tensor.matmul(dt_ps[:], wdtT[:], dtr[:, sl],
                                 start=True, stop=True)
                spe = scr.tile([C, sub], F32, tag="spe")
                nc.scalar.activation(spe[:], dt_ps[:], AF.Exp, bias=dt_b)
                nc.scalar.activation(dt_bf[:, sl], spe[:], AF.Ln, bias=1.0)
                nc.scalar.dma_start(dtv_d[:, sl], dt_bf[:, sl])
                nc.vector.tensor_tensor(v_bf[:, sl], dt_bf[:, sl], xs[:, sl],
                                        OP.mult)
                nc.scalar.dma_start(dtv_d[:, Tc + j * sub:Tc + (j + 1) * sub],
                                    v_bf[:, sl])
            ck[k] = dict(xinbf=xinbf, xs=xs, zs=zs, bB=bB, bC=bC,
                         dtv_d=dtv_d)

        def groups(k, gs):
            """Per-group interleaved scan + PE selector reduction (+D-skip)."""
            c = ck[k]
            if 0 in gs:
                y_ps = ps_y.tile([C, Tc], F32, tag="y")
                c["y_ps"] = y_ps
                for j in range(nsub):
                    sl = slice(j * sub, (j + 1) * sub)
                    nc.tensor.matmul(y_ps[:, sl], wdiag[:], c["xs"][:, sl],
                                     start=True, stop=False)
            y_ps = c["y_ps"]
            for g in gs:
                dtv = reps.tile([C, 2 * Tc], BF16, tag="dtv")
                eng = nc.scalar if (g % 2 == 1 and g < NG // 2) else nc.sync
                eng.dma_start(
                    dtv[:],
                    c["dtv_d"][8 * g:8 * g + 8, :].unsqueeze(0)
                    .broadcast_to([NST, 8, 2 * Tc]))
                dA = scanp.tile([C, Tc], BF16, tag="dA")
                nc.scalar.activation(dA[:], dtv[:, 0:Tc], AF.Exp,
                                     scale=aicol[:, g:g + 1])
                u = scanp.tile([C, Tc], BF16, tag="u")
                nc.vector.tensor_tensor(u[:], dtv[:, Tc:2 * Tc], c["bB"][:],
                                        OP.mult)
                h = hp.tile([C, Tc], BF16, tag="h")
                init = 0.0 if k == 0 else carries[g][:]
                nc.vector.tensor_tensor_scan(h[:], dA[:], u[:], init,
                                             OP.mult, OP.add)
                nc.vector.tensor_copy(carries[g][:], h[:, Tc - 1:Tc])
                nc.vector.tensor_tensor(h[:], h[:], c["bC"][:], OP.mult)
                for j in range(nsub):
                    sl = slice(j * sub, (j + 1) * sub)
                    nc.tensor.matmul(y_ps[:, sl],
                                     wsel[:, g * C:(g + 1) * C],
                                     h[:, sl],
                                     start=False, stop=(g == NG - 1))

        def gate_out(k):
            """Gate with silu(z), out_proj, residual, store."""
            c = ck.pop(k)
            t0 = k * Tc
            for j in range(nsub):
                sl = slice(j * sub, (j + 1) * sub)
                yg = scr.tile([C, sub], BF16, tag="yg")
                nc.vector.tensor_tensor(yg[:], c["y_ps"][:, sl],
                                        c["zs"][:, sl], OP.mult)
                o_ps = ps_mm.tile([C, sub], F32, tag="mmO")
                nc.tensor.matmul(o_ps[:], woutT[:], yg[:],
                                 start=True, stop=True)
                ob = io.tile([C, sub], F32, tag="ob")
                nc.scalar.copy(ob[:], o_ps[:])
                osl = slice(t0 + j * sub, t0 + (j + 1) * sub)
                nc.scalar.dma_start(y_out[:, osl], ob[:])
                nc.gpsimd.dma_start(y_out[:, osl], x_in[:, osl],
                                    accum_op=OP.add)

        # software pipeline: pre(k+1) issues mid-way through groups(k) so the
        # dt/v DRAM bounce round-trip hides under the second half of the scans
        pre(0)
        for k in range(nchunk):
            groups(k, range(0, NG // 2))
            if k + 1 < nchunk:
                pre(k + 1)
            groups(k, range(NG // 2, NG))
            gate_out(k)

    nc.insert_act_table_loads = types.MethodType(_single_act_table, nc)
    nc.compile()
    return nc


def prep_weights(ln_w, ln_b, in_proj_w, conv_w, conv_b, x_proj_w,
                 dt_proj_w, dt_proj_b, A_log, D, out_proj_w):
    wx = in_proj_w[:C] * ln_w[None, :]       # (out, in) with ln_w folded
    wz = in_proj_w[C:] * ln_w[None, :]
    xb = in_proj_w[:C] @ ln_b                # x-branch const from ln_b
    zb = in_proj_w[C:] @ ln_b
    cbx = conv_b + xb * conv_w.sum(axis=1)
    eps = np.full_like(ln_w, LN_EPS)
    cols = np.stack([dt_proj_b, D, cbx, -cbx, zb, -zb, eps],
                    axis=1).astype(np.float32)
    # selector weights: w_sel[g][p = n*8 + c8, c] = 1 iff c == 8g + c8
    wsel = np.zeros((C, NG * C), np.float32)
    for g in range(NG):
        for n in range(NST):
            for c8 in range(8):
                wsel[n * 8 + c8, g * C + 8 * g + c8] = 1.0
    # A in interleaved layout: a_icols[p = n*8 + c8, g] = -exp(A_log[8g+c8, n])
    A = -np.exp(A_log.astype(np.float32))    # (C, NST)
    aic = np.zeros((C, NG), np.float32)
    for g in range(NG):
        for n in range(NST):
            for c8 in range(8):
                aic[n * 8 + c8, g] = A[8 * g + c8, n]
    # x_proj rows reordered to [B(16), C(16), dt(8)]
    order = list(range(RANK, RANK + 2 * NST)) + list(range(RANK))
    return {
        "w_inT": np.ascontiguousarray(np.concatenate(
            [wx.T * conv_w[:, kk][None, :] for kk in range(DCONV)] + [wz.T],
            axis=1).astype(ml_dtypes.bfloat16)),
        "w_xpT": np.ascontiguousarray(
            x_proj_w[order].T.astype(ml_dtypes.bfloat16)),
        "w_dtT": np.ascontiguousarray(dt_proj_w.T.astype(ml_dtypes.bfloat16)),
        "w_outT": np.ascontiguousarray(
            out_proj_w.T.astype(ml_dtypes.bfloat16)),
        "w_sel": np.ascontiguousarray(wsel.astype(ml_dtypes.bfloat16)),
        "w_diag": np.ascontiguousarray(
            np.diag(D).astype(np.float32).astype(ml_dtypes.bfloat16)),
        "cols": cols,
        "a_icols": aic,
    }


def kernel(input, ln_w, ln_b, in_proj_w, conv_w, conv_b, x_proj_w,
           dt_proj_w, dt_proj_b, A_log, D, out_proj_w, _run=None):
    input = np.asarray(input, np.float32)
    b, c, H, W = input.shape
    L = H * W
    assert c == C and b == 8
    wts = prep_weights(
        np.asarray(ln_w, np.float32), np.asarray(ln_b, np.float32),
        np.asarray(in_proj_w, np.float32), np.asarray(conv_w, np.float32),
        np.asarray(conv_b, np.float32), np.asarray(x_proj_w, np.float32),
        np.asarray(dt_proj_w, np.float32), np.asarray(dt_proj_b, np.float32),
        np.asarray(A_log, np.float32), np.asarray(D, np.float32),
        np.asarray(out_proj_w, np.float32))
    nc = build_nc(L, 1536, 512)
    in_maps = []
    for i in range(8):
        m = {"x": np.ascontiguousarray(input[i].reshape(c, L))}
        m.update(wts)
        in_maps.append(m)
    run = _run or run_bass_kernel_spmd
    res = run(nc, in_maps, core_ids=list(range(8)))
    out = np.stack([np.asarray(res.results[i]["y"]).reshape(c, H, W)
                    for i in range(8)])
    return out.astype(np.float32)


# revision 38
# speedup vs baseline: 1.0239x; 1.0239x over previous
"""CAMMambaBlock Trainium2 kernel, v2 (state-interleaved scan layout).

Data-parallel over batch: 8 batch elements -> 8 NeuronCores. Each core runs
the full block (LayerNorm -> in_proj -> causal depthwise conv -> SiLU ->
x_proj -> dt softplus -> selective scan -> gating -> out_proj -> residual)
on its own (c=128, L=9216) slice, streaming over L in chunks of 1536.

Key layout: the selective scan runs in a state-interleaved layout.  For each
of 16 channel-groups g (8 channels each), a [128, Tc] tile holds all 16 SSM
states: partition p = n*8 + c8 carries the recurrence for (state n, channel
8g+c8).  This makes the per-state B/C coefficient broadcast a single shared
replicated tile per chunk (instead of 16 per-state broadcasts), and turns
the sum over states into PE selector-matmuls that accumulate in PSUM
(instead of a DVE/GpSimd reduction tree).

GpSimd is parked (it shares the DVE's second SBUF port; the v1 kernel's
heavy GpSimd elementwise load degraded DVE scans/TTs by 1.2-2x).  All
elementwise work is DVE in bf16 (2x mode); transcendentals on Scalar;
reductions and projections on the PE.  LayerNorm's ln_w/ln_b and the conv
bias are folded into the in_proj weights host-side.
"""
import types
import numpy as np
import ml_dtypes
from contextlib import ExitStack

import bass_rust

import concourse.bass as bass
import concourse.bacc as bacc
import concourse.tile as tile
from concourse import mybir
from concourse.bass_utils import run_bass_kernel_spmd
from concourse.hw_specs import get_activation_tables


def _single_act_table(self):
    """Force every activation onto natural_log_exp_and_others so the
    table-load pass hoists to one load."""
    if not any(i.opcode == "Activation" for i in self.all_instructions()):
        return
    keep = "natural_log_exp_and_others"
    tables = [(n, (f if n == keep else set()))
              for n, f in get_activation_tables(self.m.arch).items()]
    bass_rust.insert_act_table_loads(self, tables)


F32 = mybir.dt.float32
BF16 = mybir.dt.bfloat16
AF = mybir.ActivationFunctionType
OP = mybir.AluOpType

C = 128           # channels == d_inner == partitions
NST = 16          # SSM state dim
NG = 16           # channel groups of 8
RANK = 8          # dt rank
LN_EPS = 1e-5
DCONV = 4
HALO = 4          # halo columns at the left of `un` (col 0 unused, 1..3 conv)

L_FULL = 96 * 96  # 9216

# cols layout: [0]=dt_b [1]=D [2]=cbx [3]=-cbx [4]=zb [5]=-zb [6]=eps
I_DTB, I_D, I_CBX, I_MCBX, I_ZB, I_MZB, I_EPS = range(7)


def build_nc(L, Tc, sub=512):
    assert L % Tc == 0 and Tc % sub == 0
    nchunk = L // Tc
    nsub = Tc // sub

    nc = bacc.Bacc()
    x_in = nc.declare_dram_parameter("x", [C, L], F32, isOutput=False)
    w_inT = nc.declare_dram_parameter("w_inT", [C, 5 * C], BF16, isOutput=False)
    w_xpT = nc.declare_dram_parameter("w_xpT", [C, RANK + 2 * NST], BF16,
                                      isOutput=False)
    w_dtT = nc.declare_dram_parameter("w_dtT", [RANK, C], BF16, isOutput=False)
    w_outT = nc.declare_dram_parameter("w_outT", [C, C], BF16, isOutput=False)
    w_sel = nc.declare_dram_parameter("w_sel", [C, NG * C], BF16,
                                      isOutput=False)
    w_diag = nc.declare_dram_parameter("w_diag", [C, C], BF16, isOutput=False)
    cols = nc.declare_dram_parameter("cols", [C, 7], F32, isOutput=False)
    a_icols = nc.declare_dram_parameter("a_icols", [C, NG], F32,
                                        isOutput=False)
    y_out = nc.declare_dram_parameter("y", [C, L], F32, isOutput=True)

    with tile.TileContext(nc) as tc, ExitStack() as ctx:
        wpool = ctx.enter_context(tc.tile_pool(name="weights", bufs=1))
        state = ctx.enter_context(tc.tile_pool(name="state", bufs=1))
        io = ctx.enter_context(tc.tile_pool(name="io", bufs=2))
        work = ctx.enter_context(tc.tile_pool(name="work", bufs=2))
        scr = ctx.enter_context(tc.tile_pool(name="scratch", bufs=2))
        reps = ctx.enter_context(tc.tile_pool(name="reps", bufs=5))
        scanp = ctx.enter_context(tc.tile_pool(name="scan", bufs=3))
        hp = ctx.enter_context(tc.tile_pool(name="hp", bufs=5))
        bbp = ctx.enter_context(tc.tile_pool(name="bb", bufs=2))
        dram = ctx.enter_context(tc.tile_pool(name="dram", bufs=2,
                                              space="DRAM"))
        ps_st = ctx.enter_context(tc.tile_pool(name="ps_st", bufs=1,
                                               space="PSUM"))
        ps_mm = ctx.enter_context(tc.tile_pool(name="ps_mm", bufs=1,
                                               space="PSUM"))
        ps_y = ctx.enter_context(tc.tile_pool(name="ps_y", bufs=1,
                                              space="PSUM"))

        # ---- weights to SBUF (once) ----
        winT = wpool.tile([C, 5 * C], BF16, tag="winT")
        nc.sync.dma_start(winT[:], w_inT[:])
        wxpT = wpool.tile([C, RANK + 2 * NST], BF16, tag="wxpT")
        nc.sync.dma_start(wxpT[:], w_xpT[:])
        wdtT = wpool.tile([RANK, C], BF16, tag="wdtT")
        nc.sync.dma_start(wdtT[:], w_dtT[:])
        woutT = wpool.tile([C, C], BF16, tag="woutT")
        nc.sync.dma_start(woutT[:], w_outT[:])
        wsel = wpool.tile([C, NG * C], BF16, tag="wsel")
        nc.sync.dma_start(wsel[:], w_sel[:])
        wdiag = wpool.tile([C, C], BF16, tag="wdiag")
        nc.sync.dma_start(wdiag[:], w_diag[:])
        colsb = wpool.tile([C, 7], F32, tag="cols")
        nc.sync.dma_start(colsb[:], cols[:])
        aicol = wpool.tile([C, NG], F32, tag="aicol")
        nc.sync.dma_start(aicol[:], a_icols[:])
        ones_c = wpool.tile([C, C], BF16, tag="ones")
        nc.vector.memset(ones_c[:], 1.0 / C)

        dt_b = colsb[:, I_DTB:I_DTB + 1]
        d_col = colsb[:, I_D:I_D + 1]
        cbx = colsb[:, I_CBX:I_CBX + 1]
        mcbx = colsb[:, I_MCBX:I_MCBX + 1]
        zb = colsb[:, I_ZB:I_ZB + 1]
        mzb = colsb[:, I_MZB:I_MZB + 1]
        eps = colsb[:, I_EPS:I_EPS + 1]

        # ---- persistent scan carries, one per channel-group ----
        carries = []
        for g in range(NG):
            cr = state.tile([C, 1], BF16, tag=f"carry{g}")
            carries.append(cr)

        ck = {}        # per-chunk live tiles, keyed by chunk index
        prev_un = [None]

        def pre(k):
            """LN -> in_proj+conv -> silu -> x_proj -> dt/v -> DRAM bounce."""
            t0 = k * Tc
            xinbf = io.tile([C, Tc], BF16, tag="xinbf")
            nc.gpsimd.dma_start(xinbf[:], x_in[:, t0:t0 + Tc])

            sq = scr.tile([C, Tc], BF16, tag="sq")
            nc.scalar.activation(sq[:], xinbf[:], AF.Square)
            un = work.tile([C, Tc + HALO], BF16, tag="un")
            if k == 0:
                nc.vector.memset(un[:, 1:HALO], 0.0)
            else:
                nc.vector.tensor_copy(un[:, 1:HALO],
                                      prev_un[0][:, Tc + 1:Tc + HALO])
            prev_un[0] = un
            for j in range(nsub):
                sl = slice(j * sub, (j + 1) * sub)
                mu = ps_st.tile([C, sub], F32, tag="mu")
                nc.tensor.matmul(mu[:], ones_c[:], xinbf[:, sl],
                                 start=True, stop=True)
                m2 = ps_st.tile([C, sub], F32, tag="m2")
                nc.tensor.matmul(m2[:], ones_c[:], sq[:, sl],
                                 start=True, stop=True)
                mubf = scr.tile([C, sub], BF16, tag="mubf")
                nc.scalar.copy(mubf[:], mu[:])
                musq = scr.tile([C, sub], F32, tag="musq")
                nc.scalar.activation(musq[:], mu[:], AF.Square)
                var = scr.tile([C, sub], F32, tag="var")
                nc.vector.tensor_tensor(var[:], m2[:], musq[:], OP.subtract)
                lnv = scr.tile([C, sub], F32, tag="lnv")
                nc.scalar.activation(lnv[:], var[:], AF.Ln, bias=eps)
                rstd = scr.tile([C, sub], BF16, tag="rstd")
                nc.scalar.activation(rstd[:], lnv[:], AF.Exp, scale=-0.5)
                dmu = scr.tile([C, sub], BF16, tag="dmu")
                nc.vector.tensor_tensor(dmu[:], xinbf[:, sl], mubf[:],
                                        OP.subtract)
                nc.vector.tensor_tensor(
                    un[:, HALO + j * sub:HALO + (j + 1) * sub],
                    dmu[:], rstd[:], OP.mult)

            zs = work.tile([C, Tc], BF16, tag="zs")
            xs = work.tile([C, Tc], BF16, tag="xs")
            for j in range(nsub):
                sl = slice(j * sub, (j + 1) * sub)
                xm_ps = ps_mm.tile([C, sub], F32, tag="mmA")
                base = HALO - (DCONV - 1) + j * sub
                for kk in range(DCONV):
                    nc.tensor.matmul(
                        xm_ps[:], winT[:, kk * C:(kk + 1) * C],
                        un[:, base + kk:base + kk + sub],
                        start=(kk == 0), stop=(kk == DCONV - 1))
                z_ps = ps_mm.tile([C, sub], F32, tag="mmB")
                nc.tensor.matmul(z_ps[:], winT[:, 4 * C:5 * C],
                                 un[:, HALO + j * sub:HALO + j * sub + sub],
                                 start=True, stop=True)
                # silu(z+zb): sigmoid via exp/ln1p/exp chain, then mult
                es1 = scr.tile([C, sub], F32, tag="es1")
                nc.scalar.activation(es1[:], z_ps[:], AF.Exp, scale=-1.0,
                                     bias=mzb)
                es2 = scr.tile([C, sub], F32, tag="es2")
                nc.scalar.activation(es2[:], es1[:], AF.Ln, bias=1.0)
                sgz = scr.tile([C, sub], BF16, tag="sgz")
                nc.scalar.activation(sgz[:], es2[:], AF.Exp, scale=-1.0)
                sz = scr.tile([C, sub], BF16, tag="sz")
                nc.scalar.activation(sz[:], z_ps[:], AF.Identity, bias=zb)
                nc.vector.tensor_tensor(zs[:, sl], sz[:], sgz[:], OP.mult)
                # silu(conv + cbx)
                ec1 = scr.tile([C, sub], F32, tag="ec1")
                nc.scalar.activation(ec1[:], xm_ps[:], AF.Exp, scale=-1.0,
                                     bias=mcbx)
                ec2 = scr.tile([C, sub], F32, tag="ec2")
                nc.scalar.activation(ec2[:], ec1[:], AF.Ln, bias=1.0)
                sgc = scr.tile([C, sub], BF16, tag="sgc")
                nc.scalar.activation(sgc[:], ec2[:], AF.Exp, scale=-1.0)
                sx = scr.tile([C, sub], BF16, tag="sx")
                nc.scalar.activation(sx[:], xm_ps[:], AF.Identity, bias=cbx)
                nc.vector.tensor_tensor(xs[:, sl], sx[:], sgc[:], OP.mult)

            bc = work.tile([2 * NST, Tc], BF16, tag="bc")
            dtr = work.tile([RANK, Tc], BF16, tag="dtr")
            for j in range(nsub):
                sl = slice(j * sub, (j + 1) * sub)
                dblf = ps_mm.tile([C, sub], F32, tag="mmB")
                dbl = dblf[0:RANK + 2 * NST, :]
                nc.tensor.matmul(dbl, wxpT[:], xs[:, sl],
                                 start=True, stop=True)
                nc.scalar.copy(bc[:, sl], dblf[0:2 * NST, :])
                nc.scalar.copy(dtr[:, sl], dblf[2 * NST:2 * NST + RANK, :])
            bc_d = dram.tile([2 * NST, Tc], BF16, tag="bc_d")
            nc.scalar.dma_start(bc_d[:], bc[:])
            bB = bbp.tile([C, Tc], BF16, tag="bB")
            nc.scalar.dma_start(
                bB[:], bc_d[0:NST, :].unsqueeze(1).broadcast_to([NST, 8, Tc]))
            bC = bbp.tile([C, Tc], BF16, tag="bC")
            nc.scalar.dma_start(
                bC[:],
                bc_d[NST:2 * NST, :].unsqueeze(1).broadcast_to([NST, 8, Tc]))

            # dt = softplus(dt_proj @ dtr + dt_b); v = dt * xs; both into one
            # DRAM scratch [C, 2Tc] so each group needs a single replica DMA
            dt_bf = work.tile([C, Tc], BF16, tag="dt")
            v_bf = work.tile([C, Tc], BF16, tag="v")
            dtv_d = dram.tile([C, 2 * Tc], BF16, tag="dtv_d")
            for j in range(nsub):
                sl = slice(j * sub, (j + 1) * sub)
                dt_ps = ps_mm.tile([C, sub], F32, tag="mmA")
                nc.tensor.matmul(dt_ps[:], wdtT[:], dtr[:, sl],
                                 start=True, stop=True)
                spe = scr.tile([C, sub], F32, tag="spe")
                nc.scalar.activation(spe[:], dt_ps[:], AF.Exp, bias=dt_b)
                nc.scalar.activation(dt_bf[:, sl], spe[:], AF.Ln, bias=1.0)
                nc.scalar.dma_start(dtv_d[:, sl], dt_bf[:, sl])
                nc.vector.tensor_tensor(v_bf[:, sl], dt_bf[:, sl], xs[:, sl],
                                        OP.mult)
                nc.scalar.dma_start(dtv_d[:, Tc + j * sub:Tc + (j + 1) * sub],
                                    v_bf[:, sl])
            ck[k] = dict(xinbf=xinbf, xs=xs, zs=zs, bB=bB, bC=bC,
                         dtv_d=dtv_d)

        def groups(k, gs):
            """Per-group interleaved scan + PE selector reduction (+D-skip)."""
            c = ck[k]
            if 0 in gs:
                y_ps = ps_y.tile([C, Tc], F32, tag="y")
                c["y_ps"] = y_ps
                for j in range(nsub):
                    sl = slice(j * sub, (j + 1) * sub)
                    nc.tensor.matmul(y_ps[:, sl], wdiag[:], c["xs"][:, sl],
                                     start=True, stop=False)
            y_ps = c["y_ps"]
            for g in gs:
                dtv = reps.tile([C, 2 * Tc], BF16, tag="dtv")
                eng = nc.scalar if (g % 2 == 1 and g < NG // 2) else nc.sync
                eng.dma_start(
                    dtv[:],
                    c["dtv_d"][8 * g:8 * g + 8, :].unsqueeze(0)
                    .broadcast_to([NST, 8, 2 * Tc]))
                dA = scanp.tile([C, Tc], BF16, tag="dA")
                nc.scalar.activation(dA[:], dtv[:, 0:Tc], AF.Exp,
                                     scale=aicol[:, g:g + 1])
                u = scanp.tile([C, Tc], BF16, tag="u")
                nc.vector.tensor_tensor(u[:], dtv[:, Tc:2 * Tc], c["bB"][:],
                                        OP.mult)
                h = hp.tile([C, Tc], BF16, tag="h")
                init = 0.0 if k == 0 else carries[g][:]
                nc.vector.tensor_tensor_scan(h[:], dA[:], u[:], init,
                                             OP.mult, OP.add)
                nc.vector.tensor_copy(carries[g][:], h[:, Tc - 1:Tc])
                nc.vector.tensor_tensor(h[:], h[:], c["bC"][:], OP.mult)
                for j in range(nsub):
                    sl = slice(j * sub, (j + 1) * sub)
                    nc.tensor.matmul(y_ps[:, sl],
                                     wsel[:, g * C:(g + 1) * C],
                                     h[:, sl],
                                     start=False, stop=(g == NG - 1))

        def gate_out(k):
            """Gate with silu(z), out_proj, residual, store."""
            c = ck.pop(k)
            t0 = k * Tc
            last = (k == nchunk - 1)
            if last:
                xint = io.tile([C, Tc], F32, tag="xint")
                nc.sync.dma_start(xint[:], x_in[:, t0:t0 + Tc])
            for j in range(nsub):
                sl = slice(j * sub, (j + 1) * sub)
                yg = scr.tile([C, sub], BF16, tag="yg")
                nc.vector.tensor_tensor(yg[:], c["y_ps"][:, sl],
                                        c["zs"][:, sl], OP.mult)
                o_ps = ps_mm.tile([C, sub], F32, tag="mmO")
                nc.tensor.matmul(o_ps[:], woutT[:], yg[:],
                                 start=True, stop=True)
                ob = io.tile([C, sub], F32, tag="ob")
                osl = slice(t0 + j * sub, t0 + (j + 1) * sub)
                if last:
                    nc.vector.tensor_tensor(ob[:], o_ps[:], xint[:, sl],
                                            OP.add)
                    nc.scalar.dma_start(y_out[:, osl], ob[:])
                else:
                    nc.scalar.copy(ob[:], o_ps[:])
                    nc.scalar.dma_start(y_out[:, osl], ob[:])
                    nc.gpsimd.dma_start(y_out[:, osl], x_in[:, osl],
                                        accum_op=OP.add)

        # software pipeline: pre(k+1) issues mid-way through groups(k) so the
        # dt/v DRAM bounce round-trip hides under the second half of the scans
        pre(0)
        for k in range(nchunk):
            groups(k, range(0, NG // 2))
            if k + 1 < nchunk:
                pre(k + 1)
            groups(k, range(NG // 2, NG))
            gate_out(k)

    nc.insert_act_table_loads = types.MethodType(_single_act_table, nc)
    nc.compile()
    return nc


def prep_weights(ln_w, ln_b, in_proj_w, conv_w, conv_b, x_proj_w,
                 dt_proj_w, dt_proj_b, A_log, D, out_proj_w):
    wx = in_proj_w[:C] * ln_w[None, :]       # (out, in) with ln_w folded
    wz = in_proj_w[C:] * ln_w[None, :]
    xb = in_proj_w[:C] @ ln_b                # x-branch const from ln_b
    zb = in_proj_w[C:] @ ln_b
    cbx = conv_b + xb * conv_w.sum(axis=1)
    eps = np.full_like(ln_w, LN_EPS)
    cols = np.stack([dt_proj_b, D, cbx, -cbx, zb, -zb, eps],
                    axis=1).astype(np.float32)
    # selector weights: w_sel[g][p = n*8 + c8, c] = 1 iff c == 8g + c8
    wsel = np.zeros((C, NG * C), np.float32)
    for g in range(NG):
        for n in range(NST):
            for c8 in range(8):
                wsel[n * 8 + c8, g * C + 8 * g + c8] = 1.0
    # A in interleaved layout: a_icols[p = n*8 + c8, g] = -exp(A_log[8g+c8, n])
    A = -np.exp(A_log.astype(np.float32))    # (C, NST)
    aic = np.zeros((C, NG), np.float32)
    for g in range(NG):
        for n in range(NST):
            for c8 in range(8):
                aic[n * 8 + c8, g] = A[8 * g + c8, n]
    # x_proj rows reordered to [B(16), C(16), dt(8)]
    order = list(range(RANK, RANK + 2 * NST)) + list(range(RANK))
    return {
        "w_inT": np.ascontiguousarray(np.concatenate(
            [wx.T * conv_w[:, kk][None, :] for kk in range(DCONV)] + [wz.T],
            axis=1).astype(ml_dtypes.bfloat16)),
        "w_xpT": np.ascontiguousarray(
            x_proj_w[order].T.astype(ml_dtypes.bfloat16)),
        "w_dtT": np.ascontiguousarray(dt_proj_w.T.astype(ml_dtypes.bfloat16)),
        "w_outT": np.ascontiguousarray(
            out_proj_w.T.astype(ml_dtypes.bfloat16)),
        "w_sel": np.ascontiguousarray(wsel.astype(ml_dtypes.bfloat16)),
        "w_diag": np.ascontiguousarray(
            np.diag(D).astype(np.float32).astype(ml_dtypes.bfloat16)),
        "cols": cols,
        "a_icols": aic,
    }


def kernel(input, ln_w, ln_b, in_proj_w, conv_w, conv_b, x_proj_w,
           dt_proj_w, dt_proj_b, A_log, D, out_proj_w, _run=None):
    input = np.asarray(input, np.float32)
    b, c, H, W = input.shape
    L = H * W
    assert c == C and b == 8
    wts = prep_weights(
        np.asarray(ln_w, np.float32), np.asarray(ln_b, np.float32),
        np.asarray(in_proj_w, np.float32), np.asarray(conv_w, np.float32),
        np.asarray(conv_b, np.float32), np.asarray(x_proj_w, np.float32),
        np.asarray(dt_proj_w, np.float32), np.asarray(dt_proj_b, np.float32),
        np.asarray(A_log, np.float32), np.asarray(D, np.float32),
        np.asarray(out_proj_w, np.float32))
    nc = build_nc(L, 1536, 512)
    in_maps = []
    for i in range(8):
        m = {"x": np.ascontiguousarray(input[i].reshape(c, L))}
        m.update(wts)
        in_maps.append(m)
    run = _run or run_bass_kernel_spmd
    res = run(nc, in_maps, core_ids=list(range(8)))
    out = np.stack([np.asarray(res.results[i]["y"]).reshape(c, H, W)
                    for i in range(8)])
    return out.astype(np.float32)
